# revision 7
# baseline (speedup 1.0000x reference)
"""Trainium2 Bass kernel for packed varlen multi-head attention (AudioEncoderAttention).

Contract: kernel(**inputs) takes the FULL unsharded inputs of the reference
problem (hidden_states [8192,1024] packed as 8 sequences of 1024 tokens) and
returns the FULL output [8192,1024]. Internally the 8 sequences are sharded
one-per-NeuronCore across 8 cores (sequence parallel); every core runs the
same single-core program on its own sequence.

Per-core pipeline (T=1024 tokens, E=1024, H=16 heads, D=64):
  phase 1: q^T/k^T = W x^T (+bq via rank-1 K=1 matmul), RoPE via DMA partition
           shuffle + DVE mult/add; v = x W_v^T in [t,i] layout with an
           appended ones-column (for softmax denominators).
  phase 2: per head: S^T[t,l] = k^T.T q^T on PE (scores transposed so softmax
           needs no P^T transposes), exp on ACT straight out of PSUM (no max
           subtraction - scores are O(9) for this problem), U~[d|1, l] =
           v~^T expS accumulated on PE; denominators come out as U~ row 64.
           Normalization deferred: one batched reciprocal, broadcast via
           DRAM-bounce DMA, one mult per head pair.
  phase 3: y = attn^T.T woT + bo' (bo' = bo + wo bv, absorbing the v bias
           through the softmax's rows-sum-to-1 property).

All matmuls run bf16 operands with fp32 PSUM accumulation (fp32 PE matmuls
are 4x slower and f32r fails walrus codegen); RoPE and softmax stay fp32.
"""

import numpy as np
import ml_dtypes

import concourse.bass as bass
import concourse.mybir as mybir
import concourse.tile as tile
from concourse import bacc
from concourse.bass_utils import run_bass_kernel_spmd

F32 = mybir.dt.float32
BF16 = mybir.dt.bfloat16
AF = mybir.ActivationFunctionType
MUL = mybir.AluOpType.mult
ADD = mybir.AluOpType.add
BF = ml_dtypes.bfloat16

NCORES = 8
T = 1024          # tokens per sequence (= per core)
E = 1024          # embed dim
H = 16            # heads
D = 64            # head dim
P = 128
NE = E // P       # e-chunks (contraction)
NI = E // P       # i-chunks (qkv output channels)
NT = T // P       # t-chunks


def build_nc(loop_n=1):
    nc = bacc.Bacc("TRN2", target_bir_lowering=False, debug=False)

    xT_d = nc.dram_tensor("xT", [NE, P, T], BF16, kind="ExternalInput").ap()
    wq_d = nc.dram_tensor("wq", [NI, P, NE, P], BF16, kind="ExternalInput").ap()
    wk_d = nc.dram_tensor("wk", [NI, P, NE, P], BF16, kind="ExternalInput").ap()
    wv_d = nc.dram_tensor("wv", [P, NE, E], BF16, kind="ExternalInput").ap()
    wo_d = nc.dram_tensor("wo", [NI, P, E], BF16, kind="ExternalInput").ap()
    bq_d = nc.dram_tensor("bq", [1, E], BF16, kind="ExternalInput").ap()
    bo_d = nc.dram_tensor("bo", [1, E], BF16, kind="ExternalInput").ap()
    cos_d = nc.dram_tensor("cosT", [P, T], F32, kind="ExternalInput").ap()
    sin_d = nc.dram_tensor("sinS", [P, T], F32, kind="ExternalInput").ap()
    y_d = nc.dram_tensor("y", [T, E], F32, kind="ExternalOutput").ap()
    rscr = nc.dram_tensor("rscr", [H, T], F32, kind="Internal").ap()
    rscr2 = nc.dram_tensor("rscr2", [H, T], F32, kind="Internal").ap()

    import contextlib
    with tile.TileContext(nc) as tc:
      with (tc.For_i(0, loop_n, 1) if loop_n > 1 else contextlib.nullcontext()):
        with tc.tile_pool(name="const", bufs=1) as cpool, \
             tc.tile_pool(name="attn", bufs=1) as apool, \
             tc.tile_pool(name="qkv", bufs=1) as qpool:

            ones1 = cpool.tile([1, T], BF16, tag="ones1")
            nc.vector.memset(ones1, 1.0)
            bq_sb = cpool.tile([1, E], BF16, tag="bq")
            nc.sync.dma_start(out=bq_sb, in_=bq_d)
            bo_sb = cpool.tile([1, E], BF16, tag="bo")
            nc.sync.dma_start(out=bo_sb, in_=bo_d)
            cos_sb = cpool.tile([P, T], F32, tag="cos")
            nc.sync.dma_start(out=cos_sb, in_=cos_d)
            sin_sb = cpool.tile([P, T], F32, tag="sin")
            nc.sync.dma_start(out=sin_sb, in_=sin_d)

            attnT = apool.tile([P, NI, T], BF16, tag="attnT")

            xT = qpool.tile([P, NE, T], BF16, tag="xT")
            for ec in range(NE):
                nc.sync.dma_start(out=xT[:, ec, :], in_=xT_d[ec])
            qT = qpool.tile([P, NI, T], BF16, tag="qT")
            kT = qpool.tile([P, NI, T], BF16, tag="kT")
            vt = qpool.tile([P, NT, H, D + 1], BF16, tag="vt")
            nc.vector.memset(vt[:, :, :, D:D + 1], 1.0)

            # ---------------- phase 1: projections + RoPE -------------------
            with tc.tile_pool(name="ph1", bufs=1) as ph1, \
                 tc.tile_pool(name="psP", bufs=2, space="PSUM") as psP, \
                 tc.tile_pool(name="psV", bufs=2, space="PSUM") as psV:

                for (w_d, has_bias, dst) in ((wq_d, True, qT), (wk_d, False, kT)):
                    for ic in range(NI):
                        w_t = ph1.tile([P, NE, P], BF16, tag="wqk",
                                       name=f"w_{has_bias}_{ic}")
                        nc.sync.dma_start(out=w_t, in_=w_d[ic])
                        ps = psP.tile([P, T], F32, tag="P", name=f"psP_{ic}")
                        for th in range(2):
                            sl = slice(th * 512, (th + 1) * 512)
                            for ec in range(NE):
                                nc.tensor.matmul(ps[:, sl], w_t[:, ec, :],
                                                 xT[:, ec, sl], start=(ec == 0),
                                                 stop=(ec == NE - 1 and not has_bias))
                            if has_bias:
                                nc.tensor.matmul(ps[:, sl],
                                                 bq_sb[:, ic * P:(ic + 1) * P],
                                                 ones1[:, sl], start=False, stop=True)
                        # RoPE: raw into a staging tile (ACT), partition shuffle
                        # (DMA), then dst = raw*cos + shuf*sinS on DVE.
                        raw = ph1.tile([P, T], F32, tag="qraw")
                        nc.scalar.activation(out=raw, in_=ps, func=AF.Copy)
                        shuf = ph1.tile([P, T], F32, tag="qrot")
                        for g, src in ((0, 32), (32, 0), (64, 96), (96, 64)):
                            nc.sync.dma_start(out=shuf[g:g + 32, :],
                                              in_=raw[src:src + 32, :])
                        nc.vector.tensor_tensor(out=shuf, in0=shuf, in1=sin_sb, op=MUL)
                        nc.vector.tensor_tensor(out=raw, in0=raw, in1=cos_sb, op=MUL)
                        nc.vector.tensor_tensor(out=dst[:, ic, :], in0=raw, in1=shuf,
                                                op=ADD)

                wv_t = ph1.tile([P, NE, E], BF16, tag="wvf")
                nc.sync.dma_start(out=wv_t, in_=wv_d)
                for tcb in range(NT):
                    psv = psV.tile([P, E], F32, tag="V", name=f"psV_{tcb}")
                    for ih in range(2):
                        sl = slice(ih * 512, (ih + 1) * 512)
                        for ec in range(NE):
                            nc.tensor.matmul(psv[:, sl],
                                             xT[:, ec, tcb * P:(tcb + 1) * P],
                                             wv_t[:, ec, sl],
                                             start=(ec == 0), stop=(ec == NE - 1))
                    nc.vector.tensor_copy(
                        out=vt[:, tcb, :, 0:D],
                        in_=psv.rearrange("p (h d) -> p h d", d=D))

            # ---------------- phase 2: attention ----------------------------
            with tc.tile_pool(name="ph2", bufs=1) as ph2, \
                 tc.tile_pool(name="psS", bufs=3, space="PSUM") as psS, \
                 tc.tile_pool(name="psU", bufs=1, space="PSUM") as psU:

                for j in range(NI):          # head pair j -> heads 2j, 2j+1
                    expS = [ph2.tile([P, NT, T], BF16, tag=f"expS{ph}",
                                     name=f"expS{ph}_{j}")
                            for ph in range(2)]
                    for tcb in range(NT):
                        for ph in range(2):
                            pb = ph * 64
                            pss = psS.tile([P, T], F32, tag="S",
                                           name=f"S_{j}_{tcb}_{ph}")
                            for lc in range(2):
                                sl = slice(lc * 512, (lc + 1) * 512)
                                nc.tensor.matmul(
                                    pss[:, sl],
                                    kT[pb:pb + 64, j, tcb * P:(tcb + 1) * P],
                                    qT[pb:pb + 64, j, sl],
                                    start=True, stop=True)
                            nc.scalar.activation(out=expS[ph][:, tcb, :],
                                                 in_=pss, func=AF.Exp)
                    for ph in range(2):
                        h = 2 * j + ph
                        psu = psU.tile([D + 1, T], F32, tag="U", name=f"U_{h}")
                        for tcb in range(NT):
                            for lc in range(2):
                                sl = slice(lc * 512, (lc + 1) * 512)
                                nc.tensor.matmul(psu[:, sl], vt[:, tcb, h, :],
                                                 expS[ph][:, tcb, sl],
                                                 start=(tcb == 0),
                                                 stop=(tcb == NT - 1))
                        csrow = ph2.tile([1, T], F32, tag="csrow",
                                         name=f"csrow_{h}")
                        nc.vector.tensor_copy(out=csrow, in_=psu[D:D + 1, :])
                        nc.sync.dma_start(out=rscr[h:h + 1, :], in_=csrow)
                        nc.vector.tensor_copy(out=attnT[ph * 64:ph * 64 + 64, j, :],
                                              in_=psu[0:D, :])

                # softmax denominators: reload as [128,128], reciprocal,
                # bounce back to DRAM for partition-broadcast loads
                rc128 = ph2.tile([P, P], F32, tag="rc128")
                nc.sync.dma_start(
                    out=rc128, in_=rscr.rearrange("h (a f) -> (h a) f", f=P))
                nc.vector.reciprocal(out=rc128, in_=rc128)
                nc.sync.dma_start(
                    out=rscr2.rearrange("h (a f) -> (h a) f", f=P), in_=rc128)
                for j in range(NI):
                    rb = ph2.tile([P, T], F32, tag="rcolb", name=f"rb_{j}")
                    for ph in range(2):
                        nc.gpsimd.dma_start(
                            out=rb[ph * 64:(ph + 1) * 64, :],
                            in_=rscr2[2 * j + ph:2 * j + ph + 1, :]
                                .to_broadcast([64, T]))
                    nc.vector.tensor_tensor(out=attnT[:, j, :], in0=attnT[:, j, :],
                                            in1=rb, op=MUL)

            # ---------------- phase 3: output projection --------------------
            with tc.tile_pool(name="ph3", bufs=1) as ph3, \
                 tc.tile_pool(name="psY", bufs=3, space="PSUM") as psY:
                wo_t = [ph3.tile([P, E], BF16, tag=f"wo{icK}", name=f"wo{icK}")
                        for icK in range(NI)]
                for icK in range(NI):
                    nc.sync.dma_start(out=wo_t[icK], in_=wo_d[icK])
                for tcb in range(NT):
                    psy = psY.tile([P, E], F32, tag="Y", name=f"Y_{tcb}")
                    for jh in range(2):
                        sl = slice(jh * 512, (jh + 1) * 512)
                        for icK in range(NI):
                            nc.tensor.matmul(psy[:, sl],
                                             attnT[:, icK, tcb * P:(tcb + 1) * P],
                                             wo_t[icK][:, sl],
                                             start=(icK == 0), stop=False)
                        nc.tensor.matmul(psy[:, sl], ones1[:, 0:P], bo_sb[:, sl],
                                         start=False, stop=True)
                    yst = ph3.tile([P, E], F32, tag="yst")
                    nc.vector.tensor_copy(out=yst, in_=psy)
                    nc.sync.dma_start(out=y_d[tcb * P:(tcb + 1) * P, :], in_=yst)
    nc.compile()
    return nc


def prep_core_inputs(x_s, cos_s, sin_s, shared):
    """Per-core input dict: x_s [1024, 1024] f32, cos_s/sin_s [1024, 64]."""
    d = dict(shared)
    d["xT"] = np.ascontiguousarray(x_s.T).reshape(NE, P, T).astype(BF)
    c64 = np.ascontiguousarray(cos_s.T.astype(np.float32))    # [64, 1024]
    s64 = np.ascontiguousarray(sin_s.T.astype(np.float32))
    sS = np.concatenate([-s64[:32], s64[32:]], axis=0)        # sign folded (dest idx)
    d["cosT"] = np.concatenate([c64, c64], axis=0)
    d["sinS"] = np.concatenate([sS, sS], axis=0)
    return d


def prep_shared(wq, bq, wk, wv, bv, wo, bo):
    scale = float(D) ** -0.5
    wqT = np.ascontiguousarray((wq * scale).T)                # [e, i]
    wkT = np.ascontiguousarray(wk.T)
    wvT = np.ascontiguousarray(wv.T)
    woT = np.ascontiguousarray(wo.T)                          # [i, j]
    sh = {}
    sh["wq"] = np.ascontiguousarray(
        wqT.reshape(NE, P, NI, P).transpose(2, 1, 0, 3)).astype(BF)
    sh["wk"] = np.ascontiguousarray(
        wkT.reshape(NE, P, NI, P).transpose(2, 1, 0, 3)).astype(BF)
    sh["wv"] = np.ascontiguousarray(
        wvT.reshape(NE, P, E).transpose(1, 0, 2)).astype(BF)  # [p, ec, i]
    sh["wo"] = np.ascontiguousarray(woT.reshape(NI, P, E)).astype(BF)
    sh["bq"] = (bq * scale).reshape(1, E).astype(BF)
    sh["bo"] = (bo + wo @ bv).reshape(1, E).astype(BF)
    return sh


_NC = None


def kernel(hidden_states, cos, sin, wq, bq, wk, wv, bv, wo, bo,
           cu_seqlens, max_seqlen):
    global _NC
    hidden_states = np.asarray(hidden_states, dtype=np.float32)
    cos = np.asarray(cos, dtype=np.float32)
    sin = np.asarray(sin, dtype=np.float32)
    cu = np.asarray(cu_seqlens)
    assert hidden_states.shape == (NCORES * T, E)
    assert np.array_equal(cu, np.arange(NCORES + 1, dtype=cu.dtype) * T), \
        "kernel specialized for 8 equal sequences of 1024"

    if _NC is None:
        _NC = build_nc()
    shared = prep_shared(np.asarray(wq, np.float32), np.asarray(bq, np.float32),
                         np.asarray(wk, np.float32), np.asarray(wv, np.float32),
                         np.asarray(bv, np.float32), np.asarray(wo, np.float32),
                         np.asarray(bo, np.float32))
    in_maps = []
    for s in range(NCORES):
        sl = slice(s * T, (s + 1) * T)
        in_maps.append(prep_core_inputs(hidden_states[sl], cos[sl], sin[sl],
                                        shared))
    res = run_bass_kernel_spmd(_NC, in_maps, list(range(NCORES)))
    return np.concatenate([res.results[s]["y"] for s in range(NCORES)], axis=0)


if __name__ == "__main__":
    print("building program...")
    nc = build_nc()
    print("ok")
